# revision 1
# baseline (speedup 1.0000x reference)
"""GQA decode attention (B=16, S=4096, NH=32, NKV=8, HD=128) on 8 TRN2 cores.

Sharding: tensor-parallel over heads — 1 KV head (4 Q heads) per core.
Each core: qkv projection for its 768 wqkv rows, RoPE + QK-RMSNorm,
attention over its KV-head slice of the caches, RowParallel o_proj slice
producing a [16, 4096] partial; partials are summed on the host.

The cache scatter at last_pos is handled by baking last_pos (host-known at
compile time, compile happens inside kernel()) into the program:
 - K side: overwrite one column of the streamed K^T tile with the new
   (rope'd+normed) k before the score matmuls.
 - V side: a rank-1 correction matmul adds e_new * (v_new - v_stale) to the
   attention output accumulator.
Softmax skips max-subtraction (scores are ~N(0,1) after QK-RMSNorm); the
denominator is folded into the value matmul via a ones-column appended to V.
"""

import sys
from contextlib import ExitStack

for _p in ("/opt/trn_rl_repo",):
    if _p not in sys.path:
        sys.path.insert(0, _p)

import numpy as np

import concourse.bass as bass
import concourse.tile as tile
from concourse import mybir
from concourse.bass_utils import run_bass_kernel_spmd
from concourse.masks import make_identity

B, S, H = 16, 4096, 4096
NH, NKV, HD = 32, 8, 128
NREP = NH // NKV  # 4 q heads per kv head (= per core)
DQ = NREP * HD  # 512
NCORES = 8
EPS = 1e-5
NCH = S // 128  # 32 seq chunks
VW = 129  # V row width: 128 + 1 ones-column
F32 = mybir.dt.float32
AF = mybir.ActivationFunctionType
AX = mybir.AxisListType


def _legalize_waits(nc):
    """This walrus build accepts at most ONE sync wait on most instruction
    encodings (Matmult's S3_LW, DMA structs, ...) while Tile may attach
    several. Move excess waits onto same-engine no-ops inserted right before
    the instruction (semantically identical: the engine queue executes the
    wait no-ops, then the instruction)."""
    moved = 0
    skip = (mybir.InstNoOp, mybir.InstEventSemaphore)
    for func in nc.m.functions:
        for bb in func.blocks:
            insts = list(bb.instructions)
            out = []
            changed = False
            for inst in insts:
                si = inst.sync_info
                if (
                    si is not None
                    and si.on_wait
                    and len(si.on_wait) > 1
                    and not isinstance(inst, skip)
                ):
                    waits = list(si.on_wait)
                    for k, w in enumerate(waits[:-1]):
                        nop = mybir.InstNoOp(
                            name=f"{inst.name}-w{k}", engine=inst.engine
                        )
                        nop.sync_info = mybir.SyncInfo(on_wait=[w], on_update=[])
                        out.append(nop)
                        moved += 1
                    si.on_wait = waits[-1:]
                    inst.sync_info = si
                    changed = True
                out.append(inst)
            if changed:
                bb.instructions = out
    return moved


def _build_bass(lp, legalize=True, reps=1, vv_on_act=True, flip=False, kvbufs=3, r32=False, deep=False, ksplit=True):
    """Build the SPMD Bass program. lp: tuple of 16 ints (last_pos, baked).

    reps > 1 repeats the whole computation (for slope-based timing: the
    per-call dispatch overhead cancels between two rep counts)."""
    nc = bass.Bass("TRN2", target_bir_lowering=False, debug=False)
    R32 = mybir.dt.float32r
    rc_ = (lambda ap: ap.bitcast(R32)) if r32 else (lambda ap: ap)

    xt_d = nc.dram_tensor("xt", [128, NCH, B], F32, kind="ExternalInput")
    wq_d = nc.dram_tensor("wq", [NCH, 128, 768], F32, kind="ExternalInput")
    if ksplit:
        kt_d = nc.dram_tensor("kt", [B, 128, 2, S], mybir.dt.bfloat16, kind="ExternalInput")
    else:
        kt_d = nc.dram_tensor("kt", [B, 128, S], F32, kind="ExternalInput")
    vv_d = nc.dram_tensor("vv", [B, 128, NCH, VW], F32, kind="ExternalInput")
    ow_d = nc.dram_tensor("ow", [NREP, 128, 8, 512], F32, kind="ExternalInput")
    cosq_d = nc.dram_tensor("cosq", [B, NREP, 64], F32, kind="ExternalInput")
    sinq_d = nc.dram_tensor("sinq", [B, NREP, 64], F32, kind="ExternalInput")
    cosk_d = nc.dram_tensor("cosk", [B, 64], F32, kind="ExternalInput")
    sink_d = nc.dram_tensor("sink", [B, 64], F32, kind="ExternalInput")
    rm_d = nc.dram_tensor("rowmask", [128, B], F32, kind="ExternalInput")
    out_d = nc.dram_tensor("out_p", [B, H], F32, kind="ExternalOutput")

    with tile.TileContext(nc) as tc, ExitStack() as ctx:
        consts = ctx.enter_context(tc.tile_pool(name="consts", bufs=1))
        sb = ctx.enter_context(tc.tile_pool(name="sb", bufs=2))
        kpool = ctx.enter_context(tc.tile_pool(name="kpool", bufs=kvbufs))
        vpool = ctx.enter_context(tc.tile_pool(name="vpool", bufs=kvbufs))
        wpool = ctx.enter_context(tc.tile_pool(name="wpool", bufs=3))

        ident = consts.tile([128, 128], F32)
        make_identity(nc, ident[:, :])

        xt_sb = consts.tile([128, NCH, B], F32)
        nc.sync.dma_start(out=xt_sb[:, :, :], in_=xt_d[:, :, :])
        cosq = consts.tile([B, NREP, 64], F32)
        sinq = consts.tile([B, NREP, 64], F32)
        cosk = consts.tile([B, 64], F32)
        sink = consts.tile([B, 64], F32)
        epsq = consts.tile([B, 1], F32)
        epsk = consts.tile([B, 1], F32)
        nc.vector.memset(epsq[:, :], float(HD * EPS))
        nc.vector.memset(epsk[:, :], float(EPS))
        nc.sync.dma_start(out=cosq[:, :, :], in_=cosq_d[:, :, :])
        nc.sync.dma_start(out=sinq[:, :, :], in_=sinq_d[:, :, :])
        nc.sync.dma_start(out=cosk[:, :], in_=cosk_d[:, :])
        nc.sync.dma_start(out=sink[:, :], in_=sink_d[:, :])
        rowmask = consts.tile([128, B], F32)
        nc.sync.dma_start(out=rowmask[:, :], in_=rm_d[:, :])
        ones128 = consts.tile([128, 1], F32)
        nc.vector.memset(ones128[:, :], 1.0)

        for rep in range(reps):
            qn = consts.tile([B, NREP, 64, 2], F32)  # rope'd+normed q (with 1/sqrt(HD))
            kn = consts.tile([B, HD], F32)  # rope'd+normed k
            vn = consts.tile([B, VW], F32)  # new v row: [v_new, 1]
            enew = consts.tile([B, NREP], F32)  # exp(q . k_new / sqrt(HD))
            qT_sb = consts.tile([128, B * NREP], F32)  # col b*4+h
            oT_sb = consts.tile([128, NREP, B], F32)  # attention out, [d, (g, b)]

            # ---- qkv projection: qkv[b, o] = sum_h x[b, h] * wqkv_c[o, h] ----
            if flip:
                # stationary = weight block [h, 128 outcols], moving = xT [h, 16].
                # fp32 moving cost is per-column, so keep the moving side narrow.
                qkv_sb = consts.tile([B, 768], F32)
                with tc.tile_pool(name="psq", bufs=1, space="PSUM") as psq:
                    pst = [psq.tile([128, B], F32, name=f"qkvT{t}", tag=f"qkvT{t}") for t in range(6)]
                    for ii in range(NCH // 4):
                        wt = wpool.tile([128, 4, 768], F32, tag="wqf")
                        nc.sync.dma_start(
                            out=wt[:, :, :],
                            in_=wq_d[4 * ii:4 * ii + 4, :, :].transpose([1, 0, 2]),
                        )
                        for k in range(4):
                            i = 4 * ii + k
                            for t in range(6):
                                nc.tensor.matmul(
                                    pst[t][:, :],
                                    wt[:, k, 128 * t:128 * (t + 1)],
                                    xt_sb[:, i, :],
                                    start=(i == 0), stop=(i == NCH - 1),
                                )
                    for t in range(6):
                        tsb = sb.tile([128, B], F32, tag="qkvT_sb")
                        nc.vector.tensor_copy(tsb[:, :], pst[t][:, :])
                        trp = psq.tile([B, 128], F32, tag="trp")
                        nc.tensor.transpose(trp[:, :], tsb[:, :], ident[:, :])
                        nc.vector.tensor_copy(qkv_sb[:, 128 * t:128 * (t + 1)], trp[:, :])
                qv = qkv_sb[:, 0:DQ].rearrange("p (a b c) -> p a b c", b=64, c=2)
                q_ev, q_od = qv[:, :, :, 0], qv[:, :, :, 1]
                kv2 = qkv_sb[:, DQ:DQ + HD].rearrange("p (b c) -> p b c", c=2)
                k_ev, k_od = kv2[:, :, 0], kv2[:, :, 1]
                v_new = qkv_sb[:, DQ + HD:768]
                qkv_ps_ctx = None
            else:
                qkv_ps_ctx = tc.tile_pool(name="psq", bufs=1, space="PSUM")
                psq = qkv_ps_ctx.__enter__()
                ps_q = psq.tile([B, NREP, 64, 2], F32)
                ps_kv = psq.tile([B, 2, 64, 2], F32)
                ps_qf = ps_q[:, :, :, :].rearrange("p a b c -> p (a b c)")
                ps_kvf = ps_kv[:, :, :, :].rearrange("p a b c -> p (a b c)")
                for ii in range(NCH // 4):
                    wt = wpool.tile([128, 4, DQ], F32, tag="wqa")
                    (nc.gpsimd if deep else nc.sync).dma_start(
                        out=wt[:, :, :],
                        in_=wq_d[4 * ii:4 * ii + 4, :, 0:DQ].transpose([1, 0, 2]),
                    )
                    for k in range(4):
                        i = 4 * ii + k
                        nc.tensor.matmul(
                            ps_qf, rc_(xt_sb[:, i, :]), rc_(wt[:, k, :]),
                            start=(i == 0), stop=(i == NCH - 1),
                        )
                for ii in range(NCH // 4):
                    wt = wpool.tile([128, 4, 256], F32, tag="wqb")
                    (nc.gpsimd if deep else nc.sync).dma_start(
                        out=wt[:, :, :],
                        in_=wq_d[4 * ii:4 * ii + 4, :, DQ:768].transpose([1, 0, 2]),
                    )
                    for k in range(4):
                        i = 4 * ii + k
                        nc.tensor.matmul(
                            ps_kvf, rc_(xt_sb[:, i, :]), rc_(wt[:, k, :]),
                            start=(i == 0), stop=(i == NCH - 1),
                        )
                q_ev, q_od = ps_q[:, :, :, 0], ps_q[:, :, :, 1]
                k_ev, k_od = ps_kv[:, 0, :, 0], ps_kv[:, 0, :, 1]
                v_new = ps_kv[:, 1, :, :].rearrange("p a b -> p (a b)")

            # ---- RoPE (interleaved pairs) + QK-RMSNorm, all in [B, .] layout ----
            t0 = sb.tile([B, NREP, 64], F32, tag="t0")
            t1 = sb.tile([B, NREP, 64], F32, tag="t1")
            nc.vector.tensor_mul(t0[:, :, :], q_ev, cosq[:, :, :])
            nc.vector.tensor_mul(t1[:, :, :], q_od, sinq[:, :, :])
            nc.vector.tensor_sub(qn[:, :, :, 0], t0[:, :, :], t1[:, :, :])
            nc.vector.tensor_mul(t0[:, :, :], q_od, cosq[:, :, :])
            nc.vector.tensor_mul(t1[:, :, :], q_ev, sinq[:, :, :])
            nc.vector.tensor_add(qn[:, :, :, 1], t0[:, :, :], t1[:, :, :])

            kn2 = kn[:, :].rearrange("p (a b) -> p a b", b=2)
            t2 = sb.tile([B, 64], F32, tag="t2")
            t3 = sb.tile([B, 64], F32, tag="t3")
            nc.vector.tensor_mul(t2[:, :], k_ev, cosk[:, :])
            nc.vector.tensor_mul(t3[:, :], k_od, sink[:, :])
            nc.vector.tensor_sub(kn2[:, :, 0], t2[:, :], t3[:, :])
            nc.vector.tensor_mul(t2[:, :], k_od, cosk[:, :])
            nc.vector.tensor_mul(t3[:, :], k_ev, sink[:, :])
            nc.vector.tensor_add(kn2[:, :, 1], t2[:, :], t3[:, :])

            # new v row with ones-column (v has no rope/norm)
            nc.vector.tensor_copy(vn[:, 0:HD], v_new)
            nc.vector.memset(vn[:, HD:VW], 1.0)

            if qkv_ps_ctx is not None:
                qkv_ps_ctx.__exit__(None, None, None)

            # RMSNorm q; fold in the 1/sqrt(HD) score scale:
            # rstd' = 1/sqrt(ssq + HD*eps) = rsqrt(mean(q^2)+eps)/sqrt(HD)
            qn128 = qn[:, :, :, :].rearrange("p a b c -> p a (b c)")  # [16, 4, 128]
            sq = sb.tile([B, NREP, HD], F32, tag="sq")
            nc.vector.tensor_mul(sq[:, :, :], qn128, qn128)
            ssq = sb.tile([B, NREP, 1], F32, tag="ssq")
            nc.vector.reduce_sum(out=ssq[:, :, :], in_=sq[:, :, :], axis=AX.X)
            rstdq = sb.tile([B, NREP, 1], F32, tag="rstdq")
            nc.scalar.activation(rstdq[:, :, :], ssq[:, :, :], AF.Sqrt, bias=epsq[:, :])
            nc.vector.reciprocal(rstdq[:, :, :], rstdq[:, :, :])
            for h in range(NREP):
                nc.vector.tensor_scalar_mul(qn128[:, h, :], qn128[:, h, :], rstdq[:, h, :])

            # RMSNorm k (no extra scale)
            sk = sb.tile([B, HD], F32, tag="sk")
            nc.vector.tensor_mul(sk[:, :], kn[:, :], kn[:, :])
            ssk = sb.tile([B, 1], F32, tag="ssk")
            nc.vector.reduce_sum(out=ssk[:, :], in_=sk[:, :], axis=AX.X)
            nc.scalar.activation(ssk[:, :], ssk[:, :], AF.Sqrt, scale=1.0 / HD, bias=epsk[:, :])
            nc.vector.reciprocal(ssk[:, :], ssk[:, :])
            nc.vector.tensor_scalar_mul(kn[:, :], kn[:, :], ssk[:, :])

            # e_new[b, h] = exp(qn . kn)  (scale already folded into qn)
            prod = sb.tile([B, NREP, HD], F32, tag="prod")
            kb = kn[:, :].unsqueeze(1).broadcast_to((B, NREP, HD))
            nc.vector.tensor_mul(prod[:, :, :], qn128, kb)
            snew = sb.tile([B, NREP, 1], F32, tag="snew")
            nc.vector.reduce_sum(out=snew[:, :, :], in_=prod[:, :, :], axis=AX.X)
            nc.scalar.activation(enew[:, :].unsqueeze(2), snew[:, :, :], AF.Exp)

            # ---- transpose q to [HD, .] layout via PE ----
            with tc.tile_pool(name="psT", bufs=1, space="PSUM") as psT:
                ps_qT = psT.tile([128, NREP * B], F32)  # col h*16+b
                for h in range(NREP):
                    nc.tensor.transpose(
                        ps_qT[:, h * B:(h + 1) * B],
                        qn128[:, h, :],
                        ident[0:B, 0:B],
                    )
                # reorder h*16+b -> b*4+h while copying to SBUF
                qT_src = ps_qT[:, :].rearrange("p (h b) -> p b h", h=NREP)
                qT_dst = qT_sb[:, :].rearrange("p (b h) -> p b h", h=NREP)
                nc.vector.tensor_copy(qT_dst, qT_src)
            if ksplit:
                qT_hi = consts.tile([128, B * NREP], mybir.dt.bfloat16)
                qT_lo = consts.tile([128, B * NREP], mybir.dt.bfloat16)
                qT_rs = sb.tile([128, B * NREP], F32, tag="qTrs")
                nc.vector.tensor_copy(qT_hi[:, :], qT_sb[:, :])
                nc.vector.tensor_sub(qT_rs[:, :], qT_sb[:, :], qT_hi[:, :])
                nc.vector.tensor_copy(qT_lo[:, :], qT_rs[:, :])

            # ---- attention over the streamed caches ----
            if flip:
                den_all = consts.tile([1, B * NREP], F32)  # col b*4+h
            else:
                acc_sb = consts.tile([NREP, B, VW], F32)
                attn_n = consts.tile([NREP, B, HD], F32)
            with (
                tc.tile_pool(name="psc", bufs=(3 if deep else 2), space="PSUM") as psc_pool,
                tc.tile_pool(name="pso", bufs=2, space="PSUM") as pso_pool,
            ):
                for b in range(B):
                    pb = lp[b]
                    jb, rb = pb // 128, pb % 128
                    if ksplit:
                        kt_t = kpool.tile([128, 2, S], mybir.dt.bfloat16, tag="kt")
                        nc.sync.dma_start(out=kt_t[:, :, :], in_=kt_d[b, :, :, :])
                    else:
                        kt_t = kpool.tile([128, S], F32, tag="kt")
                        nc.sync.dma_start(out=kt_t[:, :], in_=kt_d[b, :, :])
                    vv_t = vpool.tile([128, NCH, VW], F32, tag="vv")
                    (nc.scalar if vv_on_act else nc.sync).dma_start(
                        out=vv_t[:, :, :], in_=vv_d[b, :, :, :])

                    psc = psc_pool.tile([128, 128], F32, tag="psc")
                    for j in range(NCH):
                        if ksplit:
                            sl = slice(128 * j, 128 * (j + 1))
                            nc.tensor.matmul(
                                psc[:, 4 * j:4 * j + 4], kt_t[:, 0, sl],
                                qT_hi[:, 4 * b:4 * b + 4], start=True, stop=False)
                            nc.tensor.matmul(
                                psc[:, 4 * j:4 * j + 4], kt_t[:, 0, sl],
                                qT_lo[:, 4 * b:4 * b + 4], start=False, stop=False)
                            nc.tensor.matmul(
                                psc[:, 4 * j:4 * j + 4], kt_t[:, 1, sl],
                                qT_hi[:, 4 * b:4 * b + 4], start=False, stop=True)
                        else:
                            nc.tensor.matmul(
                                psc[:, 4 * j:4 * j + 4],
                                rc_(kt_t[:, 128 * j:128 * (j + 1)]),
                                rc_(qT_sb[:, 4 * b:4 * b + 4]),
                                start=True, stop=True,
                            )
                    expt = sb.tile([128, 128], F32, tag="expt", bufs=(3 if deep else 1))
                    nc.scalar.activation(expt[:, :], psc[:, :], AF.Exp)
                    # scatter at last_pos: zero the stale position's weight; its
                    # true contribution e_new * [v_new, 1] is added back via the
                    # masked rank-1 matmuls below.
                    nc.vector.tensor_scalar_mul(
                        expt[:, 4 * jb:4 * jb + 4],
                        expt[:, 4 * jb:4 * jb + 4],
                        rowmask[:, b:b + 1],
                    )
                    # only row b of enew survives
                    enew_b = sb.tile([B, NREP], F32, tag="enewb")
                    nc.vector.tensor_scalar_mul(enew_b[:, :], enew[:, :], ident[0:B, b:b + 1])

                    if flip:
                        # stationary = V chunk [s, d], moving = exp [s, 4].
                        psoT = pso_pool.tile([128, NREP], F32, tag="psoT")
                        psd = pso_pool.tile([1, NREP], F32, tag="psd")
                        for j in range(NCH):
                            nc.tensor.matmul(
                                psoT[:, :], vv_t[:, j, 0:HD], expt[:, 4 * j:4 * j + 4],
                                start=(j == 0), stop=False,
                            )
                            nc.tensor.matmul(
                                psd[:, :], ones128[:, :], expt[:, 4 * j:4 * j + 4],
                                start=(j == 0), stop=False,
                            )
                        nc.tensor.matmul(
                            psoT[:, :], vn[:, 0:HD], enew_b[:, :],
                            start=False, stop=True,
                        )
                        nc.tensor.matmul(
                            psd[:, :], vn[:, HD:VW], enew_b[:, :],
                            start=False, stop=True,
                        )
                        nc.vector.tensor_copy(oT_sb[:, :, b], psoT[:, :])
                        nc.vector.tensor_copy(den_all[0:1, 4 * b:4 * b + 4], psd[:, :])
                    else:
                        pso = pso_pool.tile([NREP, VW], F32, tag="pso")
                        for j in range(NCH):
                            nc.tensor.matmul(
                                pso[:, :],
                                rc_(expt[:, 4 * j:4 * j + 4]),
                                rc_(vv_t[:, j, :]),
                                start=(j == 0), stop=False,
                            )
                        nc.tensor.matmul(
                            pso[:, :], rc_(enew_b[:, :]), rc_(vn[:, :]),
                            start=False, stop=True,
                        )
                        nc.vector.tensor_copy(acc_sb[:, b, :], pso[:, :])

            if flip:
                # normalize oT_sb by 1/denominator: bounce the denominators
                # through DRAM to broadcast them across partitions.
                with tc.tile_pool(name="dscr", bufs=1, space="DRAM") as dpool:
                    den_dram = dpool.tile([1, B * NREP], F32)
                    nc.sync.dma_start(out=den_dram[:, :], in_=den_all[:, :])
                    den_bc = sb.tile([128, B * NREP], F32, tag="denbc")
                    d_ap = den_dram[0, :]
                    bc_in = bass.AP(tensor=d_ap.tensor, offset=d_ap.offset,
                                    ap=[[0, 128]] + list(d_ap.ap))
                    nc.sync.dma_start(out=den_bc[:, :], in_=bc_in)
                    nc.vector.reciprocal(den_bc[:, :], den_bc[:, :])
                    rec_v = den_bc[:, :].rearrange("p (b h) -> p h b", h=NREP)
                    nc.vector.tensor_mul(oT_sb[:, :, :], oT_sb[:, :, :], rec_v)
            else:
                rec = sb.tile([NREP, B, 1], F32, tag="rec")
                nc.vector.reciprocal(rec[:, :, :], acc_sb[:, :, HD:HD + 1])
                recb = rec[:, :, :].broadcast_to((NREP, B, HD))
                nc.vector.tensor_mul(attn_n[:, :, :], acc_sb[:, :, 0:HD], recb)

            # ---- o_proj: out[b, :] = sum_g oT[:, g, b] . ow[g] ----
            with (
                tc.tile_pool(name="psT2", bufs=1, space="PSUM") as psT2,
                tc.tile_pool(name="psO", bufs=2, space="PSUM") as psO,
            ):
                if not flip:
                    ps_oT = psT2.tile([128, B * NREP], F32)  # col b*4+h
                    for b in range(B):
                        nc.tensor.transpose(
                            ps_oT[:, 4 * b:4 * b + 4],
                            attn_n[:, b, :],
                            ident[0:NREP, 0:NREP],
                        )
                    oT_src = ps_oT[:, :].rearrange("p (b h) -> p h b", h=NREP)
                    nc.vector.tensor_copy(oT_sb[:, :, :], oT_src)

                for half in range(4):
                    owts = []
                    for g in range(NREP):
                        owt = consts.tile([128, 2, 512], F32, tag=f"ow{g}")
                        (nc.gpsimd if deep else nc.sync).dma_start(
                            out=owt[:, :, :],
                            in_=ow_d[g, :, 2 * half:2 * half + 2, :],
                        )
                        owts.append(owt)
                    for k in range(2):
                        nb = 2 * half + k
                        ps_out = psO.tile([B, 512], F32, tag="po")
                        for g in range(NREP):
                            nc.tensor.matmul(
                                ps_out[:, :], rc_(oT_sb[:, g, :]), rc_(owts[g][:, k, :]),
                                start=(g == 0), stop=(g == NREP - 1),
                            )
                        out_sb = sb.tile([B, 512], F32, tag="outsb")
                        nc.vector.tensor_copy(out_sb[:, :], ps_out[:, :])
                        nc.sync.dma_start(out=out_d[:, 512 * nb:512 * (nb + 1)], in_=out_sb[:, :])

    if legalize:
        _legalize_waits(nc)
    return nc


def _prep_inputs(x, last_pos, rope_cache, wqkv, o_proj_w, cache_k, cache_v, ksplit=True):
    f32 = np.float32
    x2 = np.asarray(x, f32).reshape(B, H)
    lp = tuple(int(v) for v in np.asarray(last_pos).reshape(-1))
    rc = np.asarray(rope_cache, f32)[list(lp)]  # [16, 64, 2]
    cos, sin = rc[..., 0].copy(), rc[..., 1].copy()  # [16, 64]
    cosq = np.ascontiguousarray(np.broadcast_to(cos[:, None, :], (B, NREP, 64)))
    sinq = np.ascontiguousarray(np.broadcast_to(sin[:, None, :], (B, NREP, 64)))

    xt = np.ascontiguousarray(x2.T.reshape(NCH, 128, B).transpose(1, 0, 2))

    wqkv = np.asarray(wqkv, f32)
    o_proj_w = np.asarray(o_proj_w, f32)
    cache_k = np.asarray(cache_k, f32)
    cache_v = np.asarray(cache_v, f32)

    # [8, 16, 128, 4096] : per-core K^T
    ktall = np.ascontiguousarray(cache_k.transpose(2, 0, 3, 1))
    if ksplit:
        import ml_dtypes
        bf16 = ml_dtypes.bfloat16
        hi = ktall.astype(bf16)
        lo = (ktall - hi.astype(f32)).astype(bf16)
        ktall = np.ascontiguousarray(np.stack([hi, lo], axis=3))  # [8,16,128,2,4096]
    # [8, 16, 128, 32, 132] : per-core V with ones column, chunk-major repack
    v5 = cache_v.reshape(B, NCH, 128, NKV, HD).transpose(3, 0, 2, 1, 4)
    vvall = np.zeros((NKV, B, 128, NCH, VW), f32)
    vvall[..., :HD] = v5
    vvall[..., HD] = 1.0

    rowmask = np.ones((128, B), f32)
    for b in range(B):
        rowmask[lp[b] % 128, b] = 0.0

    per_core = []
    for c in range(NCORES):
        w_c = np.concatenate(
            [
                wqkv[c * DQ:(c + 1) * DQ],
                wqkv[NH * HD + c * HD:NH * HD + (c + 1) * HD],
                wqkv[NH * HD + NKV * HD + c * HD:NH * HD + NKV * HD + (c + 1) * HD],
            ],
            axis=0,
        )  # [768, 4096]
        wq_c = np.ascontiguousarray(w_c.T).reshape(NCH, 128, 768)
        ow_c = np.ascontiguousarray(o_proj_w[:, c * DQ:(c + 1) * DQ].T).reshape(
            NREP, 128, 8, 512
        )
        per_core.append(
            {
                "xt": xt,
                "wq": wq_c,
                "kt": ktall[c],
                "vv": vvall[c],
                "ow": ow_c,
                "cosq": cosq,
                "sinq": sinq,
                "cosk": cos,
                "sink": sin,
                "rowmask": rowmask,
            }
        )
    return lp, per_core


_NC_CACHE = {}
LAST_RESULT = None  # BassKernelResults of the most recent run (for profiling)


def kernel(**inputs):
    x = inputs["x"]
    last_pos = inputs["last_pos"]
    lp, per_core = _prep_inputs(
        x,
        last_pos,
        inputs["rope_cache"],
        inputs["wqkv"],
        inputs["o_proj_w"],
        inputs["cache_k"],
        inputs["cache_v"],
    )
    if lp not in _NC_CACHE:
        _NC_CACHE[lp] = _build_bass(lp)
    nc = _NC_CACHE[lp]
    res = run_bass_kernel_spmd(nc, per_core, core_ids=list(range(NCORES)))
    global LAST_RESULT
    LAST_RESULT = res
    results = res.results if hasattr(res, "results") else res
    out = np.zeros((B, H), np.float64)
    for c in range(NCORES):
        out += results[c]["out_p"].astype(np.float64)
    return out.astype(np.float32).reshape(B, 1, H)



# revision 3
# speedup vs baseline: 3.3056x; 3.3056x over previous
"""GQA decode attention (B=16, S=4096, NH=32, NKV=8, HD=128) on 8 TRN2 cores.

Sharding: tensor-parallel over heads — 1 KV head (4 Q heads) per core.
Each core: qkv projection for its 768 wqkv rows, RoPE + QK-RMSNorm,
attention over its KV-head slice of the caches, RowParallel o_proj slice
producing a [16, 4096] partial; partials are summed on the host.

All large operands (x, wqkv, K cache, V cache, o_proj) are stored in HBM
as bf16 — the kernel is HBM-bandwidth bound and the harness tolerance
(2e-2) has plenty of room for bf16 storage error (~0.5-1%). PSUM
accumulation stays f32 throughout.

The cache scatter at last_pos is handled by baking last_pos (host-known at
compile time, compile happens inside kernel()) into the program:
 - the stale cache position's softmax weight is zeroed via a rowmask
   multiply after the exp;
 - the true contribution e_new * [v_new, 1] is added back via a rank-1
   matmul into the attention accumulator.
Softmax skips max-subtraction (scores are ~N(0,1) after QK-RMSNorm); the
denominator is folded into the value matmul via a ones-column appended to V.
"""

import sys
from contextlib import ExitStack

for _p in ("/opt/trn_rl_repo",):
    if _p not in sys.path:
        sys.path.insert(0, _p)

import numpy as np

import concourse.bass as bass
import concourse.tile as tile
from concourse import mybir
from concourse.bass_utils import run_bass_kernel_spmd
from concourse.masks import make_identity

B, S, H = 16, 4096, 4096
NH, NKV, HD = 32, 8, 128
NREP = NH // NKV  # 4 q heads per kv head (= per core)
DQ = NREP * HD  # 512
NCORES = 8
EPS = 1e-5
NCH = S // 128  # 32 seq chunks
VW = 129  # V row width: 128 + 1 ones-column
F32 = mybir.dt.float32
BF16 = mybir.dt.bfloat16
AF = mybir.ActivationFunctionType
AX = mybir.AxisListType


def _legalize_waits(nc):
    """This walrus build accepts at most ONE sync wait on most instruction
    encodings (Matmult's S3_LW, DMA structs, ...) while Tile may attach
    several. Move excess waits onto same-engine no-ops inserted right before
    the instruction (semantically identical: the engine queue executes the
    wait no-ops, then the instruction)."""
    moved = 0
    skip = (mybir.InstNoOp, mybir.InstEventSemaphore)
    for func in nc.m.functions:
        for bb in func.blocks:
            insts = list(bb.instructions)
            out = []
            changed = False
            for inst in insts:
                si = inst.sync_info
                if (
                    si is not None
                    and si.on_wait
                    and len(si.on_wait) > 1
                    and not isinstance(inst, skip)
                ):
                    waits = list(si.on_wait)
                    for k, w in enumerate(waits[:-1]):
                        nop = mybir.InstNoOp(
                            name=f"{inst.name}-w{k}", engine=inst.engine
                        )
                        nop.sync_info = mybir.SyncInfo(on_wait=[w], on_update=[])
                        out.append(nop)
                        moved += 1
                    si.on_wait = waits[-1:]
                    inst.sync_info = si
                    changed = True
                out.append(inst)
            if changed:
                bb.instructions = out
    return moved


def _build_bass(lp, legalize=True, reps=1, kvbufs=3):
    """Build the SPMD Bass program. lp: tuple of 16 ints (last_pos, baked).

    reps > 1 repeats the whole computation (for slope-based timing: the
    per-call dispatch overhead cancels between two rep counts)."""
    nc = bass.Bass("TRN2", target_bir_lowering=False, debug=False)

    xt_d = nc.dram_tensor("xt", [128, NCH, B], BF16, kind="ExternalInput")
    wq_d = nc.dram_tensor("wq", [NCH, 128, 768], BF16, kind="ExternalInput")
    kt_d = nc.dram_tensor("kt", [B, 128, S], BF16, kind="ExternalInput")
    vv_d = nc.dram_tensor("vv", [B, 128, NCH, VW], BF16, kind="ExternalInput")
    ow_d = nc.dram_tensor("ow", [NREP, 128, 8, 512], BF16, kind="ExternalInput")
    cosq_d = nc.dram_tensor("cosq", [B, NREP, 64], F32, kind="ExternalInput")
    sinq_d = nc.dram_tensor("sinq", [B, NREP, 64], F32, kind="ExternalInput")
    cosk_d = nc.dram_tensor("cosk", [B, 64], F32, kind="ExternalInput")
    sink_d = nc.dram_tensor("sink", [B, 64], F32, kind="ExternalInput")
    rm_d = nc.dram_tensor("rowmask", [128, B], F32, kind="ExternalInput")
    out_d = nc.dram_tensor("out_p", [B, H], F32, kind="ExternalOutput")

    with tile.TileContext(nc) as tc, ExitStack() as ctx:
        consts = ctx.enter_context(tc.tile_pool(name="consts", bufs=1))
        sb = ctx.enter_context(tc.tile_pool(name="sb", bufs=2))
        kpool = ctx.enter_context(tc.tile_pool(name="kpool", bufs=kvbufs))
        vpool = ctx.enter_context(tc.tile_pool(name="vpool", bufs=kvbufs))
        wpool = ctx.enter_context(tc.tile_pool(name="wpool", bufs=3))

        ident = consts.tile([128, 128], F32)
        make_identity(nc, ident[:, :])

        xt_sb = consts.tile([128, NCH, B], BF16)
        nc.sync.dma_start(out=xt_sb[:, :, :], in_=xt_d[:, :, :])
        cosq = consts.tile([B, NREP, 64], F32)
        sinq = consts.tile([B, NREP, 64], F32)
        cosk = consts.tile([B, 64], F32)
        sink = consts.tile([B, 64], F32)
        epsq = consts.tile([B, 1], F32)
        epsk = consts.tile([B, 1], F32)
        nc.vector.memset(epsq[:, :], float(HD * EPS))
        nc.vector.memset(epsk[:, :], float(EPS))
        nc.sync.dma_start(out=cosq[:, :, :], in_=cosq_d[:, :, :])
        nc.sync.dma_start(out=sinq[:, :, :], in_=sinq_d[:, :, :])
        nc.sync.dma_start(out=cosk[:, :], in_=cosk_d[:, :])
        nc.sync.dma_start(out=sink[:, :], in_=sink_d[:, :])
        rowmask = consts.tile([128, B], F32)
        nc.sync.dma_start(out=rowmask[:, :], in_=rm_d[:, :])

        for rep in range(reps):
            qn = consts.tile([B, NREP, 64, 2], F32)  # rope'd+normed q (with 1/sqrt(HD))
            kn = consts.tile([B, HD], F32)  # rope'd+normed k
            vn = consts.tile([B, VW], BF16)  # new v row: [v_new, 1]
            enew = consts.tile([B, NREP], BF16)  # exp(q . k_new / sqrt(HD))
            qT_bf = consts.tile([128, B * NREP], BF16)  # col b*4+h
            oT_sb = consts.tile([128, NREP, B], BF16)  # attention out, [d, (g, b)]

            # ---- qkv projection: qkv[b, o] = sum_h x[b, h] * wqkv_c[o, h] ----
            qkv_ps_ctx = tc.tile_pool(name="psq", bufs=1, space="PSUM")
            psq = qkv_ps_ctx.__enter__()
            ps_q = psq.tile([B, NREP, 64, 2], F32)
            ps_kv = psq.tile([B, 2, 64, 2], F32)
            ps_qf = ps_q[:, :, :, :].rearrange("p a b c -> p (a b c)")
            ps_kvf = ps_kv[:, :, :, :].rearrange("p a b c -> p (a b c)")
            for ii in range(NCH // 4):
                wt = wpool.tile([128, 4, DQ], BF16, tag="wqa")
                nc.sync.dma_start(
                    out=wt[:, :, :],
                    in_=wq_d[4 * ii:4 * ii + 4, :, 0:DQ].transpose([1, 0, 2]),
                )
                for k in range(4):
                    i = 4 * ii + k
                    nc.tensor.matmul(
                        ps_qf, xt_sb[:, i, :], wt[:, k, :],
                        start=(i == 0), stop=(i == NCH - 1),
                    )
            for ii in range(NCH // 4):
                wt = wpool.tile([128, 4, 256], BF16, tag="wqb")
                nc.sync.dma_start(
                    out=wt[:, :, :],
                    in_=wq_d[4 * ii:4 * ii + 4, :, DQ:768].transpose([1, 0, 2]),
                )
                for k in range(4):
                    i = 4 * ii + k
                    nc.tensor.matmul(
                        ps_kvf, xt_sb[:, i, :], wt[:, k, :],
                        start=(i == 0), stop=(i == NCH - 1),
                    )
            q_ev, q_od = ps_q[:, :, :, 0], ps_q[:, :, :, 1]
            k_ev, k_od = ps_kv[:, 0, :, 0], ps_kv[:, 0, :, 1]
            v_new = ps_kv[:, 1, :, :].rearrange("p a b -> p (a b)")

            # ---- RoPE (interleaved pairs) + QK-RMSNorm, all in [B, .] layout ----
            t0 = sb.tile([B, NREP, 64], F32, tag="t0")
            t1 = sb.tile([B, NREP, 64], F32, tag="t1")
            nc.vector.tensor_mul(t0[:, :, :], q_ev, cosq[:, :, :])
            nc.vector.tensor_mul(t1[:, :, :], q_od, sinq[:, :, :])
            nc.vector.tensor_sub(qn[:, :, :, 0], t0[:, :, :], t1[:, :, :])
            nc.vector.tensor_mul(t0[:, :, :], q_od, cosq[:, :, :])
            nc.vector.tensor_mul(t1[:, :, :], q_ev, sinq[:, :, :])
            nc.vector.tensor_add(qn[:, :, :, 1], t0[:, :, :], t1[:, :, :])

            kn2 = kn[:, :].rearrange("p (a b) -> p a b", b=2)
            t2 = sb.tile([B, 64], F32, tag="t2")
            t3 = sb.tile([B, 64], F32, tag="t3")
            nc.vector.tensor_mul(t2[:, :], k_ev, cosk[:, :])
            nc.vector.tensor_mul(t3[:, :], k_od, sink[:, :])
            nc.vector.tensor_sub(kn2[:, :, 0], t2[:, :], t3[:, :])
            nc.vector.tensor_mul(t2[:, :], k_od, cosk[:, :])
            nc.vector.tensor_mul(t3[:, :], k_ev, sink[:, :])
            nc.vector.tensor_add(kn2[:, :, 1], t2[:, :], t3[:, :])

            # new v row with ones-column (v has no rope/norm)
            nc.vector.tensor_copy(vn[:, 0:HD], v_new)
            nc.vector.memset(vn[:, HD:VW], 1.0)

            qkv_ps_ctx.__exit__(None, None, None)

            # RMSNorm q; fold in the 1/sqrt(HD) score scale:
            # rstd' = 1/sqrt(ssq + HD*eps) = rsqrt(mean(q^2)+eps)/sqrt(HD)
            qn128 = qn[:, :, :, :].rearrange("p a b c -> p a (b c)")  # [16, 4, 128]
            sq = sb.tile([B, NREP, HD], F32, tag="sq")
            nc.vector.tensor_mul(sq[:, :, :], qn128, qn128)
            ssq = sb.tile([B, NREP, 1], F32, tag="ssq")
            nc.vector.reduce_sum(out=ssq[:, :, :], in_=sq[:, :, :], axis=AX.X)
            rstdq = sb.tile([B, NREP, 1], F32, tag="rstdq")
            nc.scalar.activation(rstdq[:, :, :], ssq[:, :, :], AF.Sqrt, bias=epsq[:, :])
            nc.vector.reciprocal(rstdq[:, :, :], rstdq[:, :, :])
            for h in range(NREP):
                nc.vector.tensor_scalar_mul(qn128[:, h, :], qn128[:, h, :], rstdq[:, h, :])

            # RMSNorm k (no extra scale)
            sk = sb.tile([B, HD], F32, tag="sk")
            nc.vector.tensor_mul(sk[:, :], kn[:, :], kn[:, :])
            ssk = sb.tile([B, 1], F32, tag="ssk")
            nc.vector.reduce_sum(out=ssk[:, :], in_=sk[:, :], axis=AX.X)
            nc.scalar.activation(ssk[:, :], ssk[:, :], AF.Sqrt, scale=1.0 / HD, bias=epsk[:, :])
            nc.vector.reciprocal(ssk[:, :], ssk[:, :])
            nc.vector.tensor_scalar_mul(kn[:, :], kn[:, :], ssk[:, :])

            # e_new[b, h] = exp(qn . kn)  (scale already folded into qn)
            prod = sb.tile([B, NREP, HD], F32, tag="prod")
            kb = kn[:, :].unsqueeze(1).broadcast_to((B, NREP, HD))
            nc.vector.tensor_mul(prod[:, :, :], qn128, kb)
            snew = sb.tile([B, NREP, 1], F32, tag="snew")
            nc.vector.reduce_sum(out=snew[:, :, :], in_=prod[:, :, :], axis=AX.X)
            nc.scalar.activation(enew[:, :].unsqueeze(2), snew[:, :, :], AF.Exp)

            # ---- transpose q to [HD, .] layout via PE; cast to bf16 ----
            with tc.tile_pool(name="psT", bufs=1, space="PSUM") as psT:
                ps_qT = psT.tile([128, NREP * B], F32)  # col h*16+b
                for h in range(NREP):
                    nc.tensor.transpose(
                        ps_qT[:, h * B:(h + 1) * B],
                        qn128[:, h, :],
                        ident[0:B, 0:B],
                    )
                # reorder h*16+b -> b*4+h while copying to SBUF (casts to bf16)
                qT_src = ps_qT[:, :].rearrange("p (h b) -> p b h", h=NREP)
                qT_dst = qT_bf[:, :].rearrange("p (b h) -> p b h", h=NREP)
                nc.vector.tensor_copy(qT_dst, qT_src)

            # ---- attention over the streamed caches ----
            acc_sb = consts.tile([NREP, B, VW], F32)
            attn_n = consts.tile([NREP, B, HD], F32)
            with (
                tc.tile_pool(name="psc", bufs=2, space="PSUM") as psc_pool,
                tc.tile_pool(name="pso", bufs=2, space="PSUM") as pso_pool,
            ):
                for b in range(B):
                    pb = lp[b]
                    jb, rb = pb // 128, pb % 128
                    kt_t = kpool.tile([128, S], BF16, tag="kt")
                    nc.sync.dma_start(out=kt_t[:, :], in_=kt_d[b, :, :])
                    vv_t = vpool.tile([128, NCH, VW], BF16, tag="vv")
                    nc.scalar.dma_start(out=vv_t[:, :, :], in_=vv_d[b, :, :, :])

                    psc = psc_pool.tile([128, 128], F32, tag="psc")
                    for j in range(NCH):
                        nc.tensor.matmul(
                            psc[:, 4 * j:4 * j + 4],
                            kt_t[:, 128 * j:128 * (j + 1)],
                            qT_bf[:, 4 * b:4 * b + 4],
                            start=True, stop=True,
                        )
                    expt = sb.tile([128, 128], BF16, tag="expt")
                    nc.scalar.activation(expt[:, :], psc[:, :], AF.Exp)
                    # scatter at last_pos: zero the stale position's weight; its
                    # true contribution e_new * [v_new, 1] is added back via the
                    # masked rank-1 matmuls below.
                    nc.vector.tensor_scalar_mul(
                        expt[:, 4 * jb:4 * jb + 4],
                        expt[:, 4 * jb:4 * jb + 4],
                        rowmask[:, b:b + 1],
                    )
                    # only row b of enew survives
                    enew_b = sb.tile([B, NREP], BF16, tag="enewb")
                    nc.vector.tensor_scalar_mul(enew_b[:, :], enew[:, :], ident[0:B, b:b + 1])

                    pso = pso_pool.tile([NREP, VW], F32, tag="pso")
                    for j in range(NCH):
                        nc.tensor.matmul(
                            pso[:, :],
                            expt[:, 4 * j:4 * j + 4],
                            vv_t[:, j, :],
                            start=(j == 0), stop=False,
                        )
                    nc.tensor.matmul(
                        pso[:, :], enew_b[:, :], vn[:, :],
                        start=False, stop=True,
                    )
                    nc.vector.tensor_copy(acc_sb[:, b, :], pso[:, :])

            rec = sb.tile([NREP, B, 1], F32, tag="rec")
            nc.vector.reciprocal(rec[:, :, :], acc_sb[:, :, HD:HD + 1])
            recb = rec[:, :, :].broadcast_to((NREP, B, HD))
            nc.vector.tensor_mul(attn_n[:, :, :], acc_sb[:, :, 0:HD], recb)

            # ---- o_proj: out[b, :] = sum_g oT[:, g, b] . ow[g] ----
            with (
                tc.tile_pool(name="psT2", bufs=1, space="PSUM") as psT2,
                tc.tile_pool(name="psO", bufs=2, space="PSUM") as psO,
            ):
                ps_oT = psT2.tile([128, B * NREP], F32)  # col b*4+h
                for b in range(B):
                    nc.tensor.transpose(
                        ps_oT[:, 4 * b:4 * b + 4],
                        attn_n[:, b, :],
                        ident[0:NREP, 0:NREP],
                    )
                oT_src = ps_oT[:, :].rearrange("p (b h) -> p h b", h=NREP)
                nc.vector.tensor_copy(oT_sb[:, :, :], oT_src)

                for half in range(4):
                    owts = []
                    for g in range(NREP):
                        owt = consts.tile([128, 2, 512], BF16, tag=f"ow{g}")
                        nc.sync.dma_start(
                            out=owt[:, :, :],
                            in_=ow_d[g, :, 2 * half:2 * half + 2, :],
                        )
                        owts.append(owt)
                    for k in range(2):
                        nb = 2 * half + k
                        ps_out = psO.tile([B, 512], F32, tag="po")
                        for g in range(NREP):
                            nc.tensor.matmul(
                                ps_out[:, :], oT_sb[:, g, :], owts[g][:, k, :],
                                start=(g == 0), stop=(g == NREP - 1),
                            )
                        out_sb = sb.tile([B, 512], F32, tag="outsb")
                        nc.vector.tensor_copy(out_sb[:, :], ps_out[:, :])
                        nc.sync.dma_start(out=out_d[:, 512 * nb:512 * (nb + 1)], in_=out_sb[:, :])

    if legalize:
        _legalize_waits(nc)
    return nc


def _prep_inputs(x, last_pos, rope_cache, wqkv, o_proj_w, cache_k, cache_v):
    import ml_dtypes
    f32 = np.float32
    bf16 = ml_dtypes.bfloat16
    x2 = np.asarray(x, f32).reshape(B, H)
    lp = tuple(int(v) for v in np.asarray(last_pos).reshape(-1))
    rc = np.asarray(rope_cache, f32)[list(lp)]  # [16, 64, 2]
    cos, sin = rc[..., 0].copy(), rc[..., 1].copy()  # [16, 64]
    cosq = np.ascontiguousarray(np.broadcast_to(cos[:, None, :], (B, NREP, 64)))
    sinq = np.ascontiguousarray(np.broadcast_to(sin[:, None, :], (B, NREP, 64)))

    xt = np.ascontiguousarray(x2.T.reshape(NCH, 128, B).transpose(1, 0, 2)).astype(bf16)

    wqkv = np.asarray(wqkv, f32)
    o_proj_w = np.asarray(o_proj_w, f32)
    cache_k = np.asarray(cache_k, f32)
    cache_v = np.asarray(cache_v, f32)

    # [8, 16, 128, 4096] : per-core K^T, bf16
    ktall = np.ascontiguousarray(cache_k.transpose(2, 0, 3, 1)).astype(bf16)
    # [8, 16, 128, 32, 129] : per-core V with ones column, chunk-major repack
    v5 = cache_v.reshape(B, NCH, 128, NKV, HD).transpose(3, 0, 2, 1, 4)
    vvall = np.zeros((NKV, B, 128, NCH, VW), bf16)
    vvall[..., :HD] = v5.astype(bf16)
    vvall[..., HD] = 1.0

    rowmask = np.ones((128, B), f32)
    for b in range(B):
        rowmask[lp[b] % 128, b] = 0.0

    per_core = []
    for c in range(NCORES):
        w_c = np.concatenate(
            [
                wqkv[c * DQ:(c + 1) * DQ],
                wqkv[NH * HD + c * HD:NH * HD + (c + 1) * HD],
                wqkv[NH * HD + NKV * HD + c * HD:NH * HD + NKV * HD + (c + 1) * HD],
            ],
            axis=0,
        )  # [768, 4096]
        wq_c = np.ascontiguousarray(w_c.T).reshape(NCH, 128, 768).astype(bf16)
        ow_c = np.ascontiguousarray(o_proj_w[:, c * DQ:(c + 1) * DQ].T).reshape(
            NREP, 128, 8, 512
        ).astype(bf16)
        per_core.append(
            {
                "xt": xt,
                "wq": wq_c,
                "kt": ktall[c],
                "vv": vvall[c],
                "ow": ow_c,
                "cosq": cosq,
                "sinq": sinq,
                "cosk": cos,
                "sink": sin,
                "rowmask": rowmask,
            }
        )
    return lp, per_core


_NC_CACHE = {}
LAST_RESULT = None  # BassKernelResults of the most recent run (for profiling)


def kernel(**inputs):
    x = inputs["x"]
    last_pos = inputs["last_pos"]
    lp, per_core = _prep_inputs(
        x,
        last_pos,
        inputs["rope_cache"],
        inputs["wqkv"],
        inputs["o_proj_w"],
        inputs["cache_k"],
        inputs["cache_v"],
    )
    if lp not in _NC_CACHE:
        _NC_CACHE[lp] = _build_bass(lp)
    nc = _NC_CACHE[lp]
    res = run_bass_kernel_spmd(nc, per_core, core_ids=list(range(NCORES)))
    global LAST_RESULT
    LAST_RESULT = res
    results = res.results if hasattr(res, "results") else res
    out = np.zeros((B, H), np.float64)
    for c in range(NCORES):
        out += results[c]["out_p"].astype(np.float64)
    return out.astype(np.float32).reshape(B, 1, H)
